# revision 23
# baseline (speedup 1.0000x reference)
"""Multi-head attention TRN2 kernel (B=2, S=2048, D=1024, H=16).

Sharding (8 cores): B(2) x head-group(2) x query-block(2).
Each core: one batch b, 8 heads, 1024 query rows. The output projection
is a per-head-group partial sum; the host adds the two partials while
gathering (unshard step).

On-chip layout is feature-major: activations are [feature, token] so
every matmul contracts along the partition dim. Host sharding hands each
core pre-transposed contiguous arrays in bf16 (the kernel computes in
bf16 anyway; converting on host removes the on-chip cast phase and
halves HBM traffic).

Softmax: the reference masks scores with -1e-9 (sic), so masked
positions contribute exp(-1e-9) == 1.0f exactly. We multiply the raw
scores by the 0/1 mask in PSUM (masked -> exp(0) == 1, identical
result), so exp(s*m/8) is directly the PV weight; an appended
ones-column on V yields the softmax denominator in the same matmul.
"""

import sys

if "/opt/trn_rl_repo" not in sys.path:
    sys.path.insert(0, "/opt/trn_rl_repo")

import numpy as np
import ml_dtypes

import concourse.bass as bass
import concourse.tile as tile
from concourse import bacc, mybir
from concourse.bass_utils import run_bass_kernel_spmd

F32 = mybir.dt.float32
F32R = mybir.dt.float32r
BF16 = mybir.dt.bfloat16
AF = mybir.ActivationFunctionType
ALU = mybir.AluOpType

B, S, D, H = 2, 2048, 1024, 16
DK = 64
Q = 1024          # query rows per core
DH = 512          # head-group feature dims per core
NPAIR = 4         # head pairs per core
KC = S // 128     # 16 contraction chunks over k tokens
EC = D // 128     # 8 contraction chunks over model dim
QNB = Q // 512    # 2 query n-blocks
SNB = S // 512    # 4 khT n-blocks
HC = DH // 128    # 4 xT partition chunks

_PROGRAM = None


def _build_program():
    nc = bacc.Bacc("TRN2", debug=False, num_devices=8)

    # all host-side arrays are pre-folded to [128 partitions, ...] so every
    # DMA is a flat contiguous 2D transfer (minimal descriptor count)
    qT = nc.dram_tensor("qT", [128, EC, Q], BF16, kind="ExternalInput")
    kT = nc.dram_tensor("kT", [128, SNB, EC, 512], BF16, kind="ExternalInput")
    vT = nc.dram_tensor("vT", [128, 4, EC, 512], BF16, kind="ExternalInput")
    maskT = nc.dram_tensor("maskT", [128, KC, Q], BF16, kind="ExternalInput")
    wqT = nc.dram_tensor("wqT", [128, EC, DH], BF16, kind="ExternalInput")
    wkT = nc.dram_tensor("wkT", [128, EC, DH], BF16, kind="ExternalInput")
    wvT = nc.dram_tensor("wvT", [128, EC, DH], BF16, kind="ExternalInput")
    woT = nc.dram_tensor("woT", [128, HC, D], BF16, kind="ExternalInput")
    bqv = nc.dram_tensor("bqv", [128, NPAIR], F32, kind="ExternalInput")
    bkv = nc.dram_tensor("bkv", [128, NPAIR], F32, kind="ExternalInput")
    bov = nc.dram_tensor("bov", [128, EC], F32, kind="ExternalInput")
    out = nc.dram_tensor("out", [D, Q], F32, kind="ExternalOutput")

    with tile.TileContext(nc) as tc:
        _emit(nc, tc, qT, kT, vT, maskT, wqT, wkT, wvT, woT, bqv, bkv, bov, out)
    nc.compile()
    return nc


def _emit(nc, tc, qT, kT, vT, maskT, wqT, wkT, wvT, woT, bqv, bkv, bov, out):
    from contextlib import ExitStack

    ctx = ExitStack()
    with ctx:
        consts = ctx.enter_context(tc.tile_pool(name="consts", bufs=1))
        big = ctx.enter_context(tc.tile_pool(name="big", bufs=1))
        work = ctx.enter_context(tc.tile_pool(name="work", bufs=2))
        pp = ctx.enter_context(tc.tile_pool(name="pp", bufs=2, space="PSUM"))
        pvp = ctx.enter_context(tc.tile_pool(name="pvp", bufs=1, space="PSUM"))
        scp = ctx.enter_context(tc.tile_pool(name="scp", bufs=4, space="PSUM"))
        kqd_cm = tc.tile_pool(name="kqd", bufs=1, side="right")
        kqd = kqd_cm.__enter__()

        # ---- small constants ----
        t_bq = consts.tile([128, NPAIR], F32)
        t_bk = consts.tile([128, NPAIR], F32)
        t_bo = consts.tile([128, EC], F32)
        nc.sync.dma_start(out=t_bq, in_=bqv[:, :])
        nc.sync.dma_start(out=t_bk, in_=bkv[:, :])
        nc.sync.dma_start(out=t_bo, in_=bov[:, :])

        # ---- resident activation tensors ----
        khT = [big.tile([128, S], BF16, name=f"khT{i}") for i in range(NPAIR)]
        qhT = [big.tile([128, Q], BF16, name=f"qhT{i}") for i in range(NPAIR)]
        vh_aug = [big.tile([128, 8, 65], BF16, name=f"vha{i}") for i in range(KC)]
        mbf = [big.tile([128, 4, Q], BF16, name=f"mbf{i}") for i in range(4)]
        mb = [mbf[i // 4][:, i % 4, :] for i in range(KC)]

        # ---- k/q weights + inputs: few BIG multi-dim DMAs (queue issue
        # time, ~0.6us per dma_start, was the startup limiter) ----
        kTf = kqd.tile([128, SNB, EC, 512], BF16, name="kTf")
        qTf = kqd.tile([128, EC, Q], BF16, name="qTf")
        wkf = kqd.tile([128, EC, DH], BF16, name="wkf")
        wqf = kqd.tile([128, EC, DH], BF16, name="wqf")
        wkb = [wkf[:, i, :] for i in range(EC)]
        wqb = [wqf[:, i, :] for i in range(EC)]

        nc.sync.dma_start(out=wkf, in_=wkT[:, :, :])
        # kT loaded in 512-col blocks so the first khps proj tile
        # unblocks after ~1MB
        for sb in range(SNB):
            nc.sync.dma_start(out=kTf[:, sb], in_=kT[:, sb])
        nc.sync.dma_start(out=wqf, in_=wqT[:, :, :])
        nc.sync.dma_start(out=qTf, in_=qT[:, :, :])

        def proj_tile(p, j):
            if j < SNB:
                sb = j
                ps = pp.tile([128, 512], F32, tag="pp", name=f"khps{p}_{sb}")
                for ec in range(EC):
                    nc.tensor.matmul(
                        ps[:, :], wkb[ec][:, p * 128:(p + 1) * 128],
                        kTf[:, sb, ec, :],
                        start=(ec == 0), stop=(ec == EC - 1))
                nc.vector.tensor_scalar(
                    out=khT[p][:, sb * 512:(sb + 1) * 512], in0=ps[:, :],
                    scalar1=t_bk[:, p:p + 1], scalar2=None, op0=ALU.add)
            else:
                qb = j - SNB
                ps = pp.tile([128, 512], F32, tag="pp", name=f"qhps{p}_{qb}")
                for ec in range(EC):
                    nc.tensor.matmul(
                        ps[:, :], wqb[ec][:, p * 128:(p + 1) * 128],
                        qTf[:, ec, qb * 512:(qb + 1) * 512],
                        start=(ec == 0), stop=(ec == EC - 1))
                nc.vector.tensor_scalar(
                    out=qhT[p][:, qb * 512:(qb + 1) * 512], in0=ps[:, :],
                    scalar1=t_bq[:, p:p + 1], scalar2=None, op0=ALU.add)

        def proj_pair(p):
            for j in range(SNB + QNB):
                proj_tile(p, j)

        proj_pair(0)

        # ---- masks: 4 consolidated bf16 DMAs on the sync queue ----
        for g in range(4):
            nc.sync.dma_start(out=mbf[g], in_=maskT[:, 4 * g:4 * (g + 1), :])

        # ---- vh projection setup (interleaved with first attention pass) ----
        vs = ctx.enter_context(tc.tile_pool(name="vs", bufs=2))
        vtbp = ctx.enter_context(tc.tile_pool(name="vtb", bufs=1))
        wvf = vtbp.tile([128, EC, DH], BF16, name="wvf")
        wvb = [wvf[:, i, :] for i in range(EC)]
        nc.gpsimd.dma_start(out=wvf, in_=wvT[:, :, :])
        for sc in range(KC):
            nc.vector.memset(vh_aug[sc][:, :, 64:65], 1.0)

        def emit_vproj_qtr(qtr):
            vqf = vs.tile([128, EC, 512], BF16, tag="vq", name=f"vq{qtr}")
            vq = [vqf[:, i, :] for i in range(EC)]
            nc.gpsimd.dma_start(out=vqf, in_=vT[:, qtr])
            for si in range(4):
                sc = qtr * 4 + si
                ps = pp.tile([128, 512], F32, tag="pp", name=f"vps{sc}")
                for ec in range(EC):
                    nc.tensor.matmul(
                        ps[:, :], vq[ec][:, si * 128:(si + 1) * 128], wvb[ec][:, :],
                        start=(ec == 0), stop=(ec == EC - 1))
                nc.scalar.copy(
                    out=vh_aug[sc][:, :, 0:64],
                    in_=ps.rearrange("p (h d) -> p h d", h=8))

        # ---- attention ----
        xT = [big.tile([128, Q], BF16, name=f"xT{i}") for i in range(HC)]
        LAG = 2
        if True:
            pending_tail = [None]

            def run_iteration(p, nb, vproj=False, proj_next=False):
                pv0 = pvp.tile([65, 512], F32, tag="pv0", name=f"pv0_{p}{nb}")
                pv1 = pvp.tile([65, 512], F32, tag="pv1", name=f"pv1_{p}{nb}")
                Etiles = {}

                def emit_scores(kc):
                    sc0 = scp.tile([128, 512], F32, tag="sc",
                                   name=f"sc0_{p}_{nb}_{kc}")
                    sc1 = scp.tile([128, 512], F32, tag="sc",
                                   name=f"sc1_{p}_{nb}_{kc}")
                    nc.tensor.matmul(
                        sc0[:, :],
                        khT[p][0:64, kc * 128:(kc + 1) * 128],
                        qhT[p][0:64, nb * 512:(nb + 1) * 512],
                        start=True, stop=True)
                    nc.tensor.matmul(
                        sc1[:, :],
                        khT[p][64:128, kc * 128:(kc + 1) * 128],
                        qhT[p][64:128, nb * 512:(nb + 1) * 512],
                        start=True, stop=True, tile_position=(64, 0))
                    # mask in place: masked scores -> 0 -> exp(0) == 1,
                    # matching the reference's exp(-1e-9) == 1.0f
                    m_in = mb[kc][:, nb * 512:(nb + 1) * 512]
                    for hh, sc_ps in ((0, sc0), (1, sc1)):
                        nc.vector.tensor_tensor(
                            out=sc_ps[:, :], in0=sc_ps[:, :], in1=m_in,
                            op=ALU.mult)
                        E = work.tile([128, 512], BF16, tag="E", bufs=6,
                                      name=f"E{p}_{nb}_{kc}_{hh}")
                        nc.scalar.activation(
                            out=E, in_=sc_ps[:, :], func=AF.Exp, scale=0.125)
                        Etiles[(kc, hh)] = E

                def emit_pv(kc):
                    for hh in range(2):
                        nc.tensor.matmul(
                            (pv0, pv1)[hh][:, :], vh_aug[kc][:, 2 * p + hh, :],
                            Etiles.pop((kc, hh))[:, :],
                            start=(kc == 0), stop=(kc == KC - 1))

                for kc in range(KC + LAG):
                    if vproj and kc < KC and kc % 4 == 0:
                        emit_vproj_qtr(kc // 4)
                    if proj_next and kc >= 2 and kc % 2 == 0 and (kc - 2) // 2 < 6:
                        proj_tile(p + 1, (kc - 2) // 2)
                    if kc < KC:
                        emit_scores(kc)
                    if kc == 3 and pending_tail[0] is not None:
                        # previous iteration's tail drops into the DVE/GpSimd
                        # queues BEHIND this iteration's first mask ops
                        pending_tail[0]()
                        pending_tail[0] = None
                    if kc >= LAG:
                        emit_pv(kc - LAG)

                # evacuate PSUM immediately (ACT) so the pv banks free fast;
                # the rest of the tail is deferred into the next iteration
                pvns = []
                for hh, pv in ((0, pv0), (1, pv1)):
                    pvn = work.tile([65, 512], F32, tag="pvn", bufs=2,
                                    name=f"pvn{p}{nb}{hh}")
                    nc.scalar.copy(out=pvn, in_=pv[:, :])
                    pvns.append(pvn)

                def tail():
                    rcps = []
                    for hh in range(2):
                        den = work.tile([1, 512], F32, tag="den", bufs=2,
                                        name=f"den{p}{nb}{hh}")
                        nc.vector.tensor_copy(out=den, in_=pvns[hh][64:65, :])
                        rcp_f = work.tile([1, 512], F32, tag="rcpf", bufs=2,
                                          name=f"rcpf{p}{nb}{hh}")
                        nc.vector.reciprocal_approx_fast(out=rcp_f, in_=den)
                        rcps.append(rcp_f)
                    brs = []
                    for hh in range(2):
                        br_sb = work.tile([64, 512], F32, tag="brs", bufs=2,
                                          name=f"brs{p}{nb}{hh}")
                        nc.gpsimd.partition_broadcast(br_sb, rcps[hh])
                        brs.append(br_sb)
                    for hh in range(2):
                        h = 2 * p + hh
                        nc.vector.tensor_tensor(
                            out=xT[h // 2][(h % 2) * 64:(h % 2) * 64 + 64,
                                           nb * 512:(nb + 1) * 512],
                            in0=pvns[hh][0:64, :], in1=brs[hh], op=ALU.mult)

                pending_tail[0] = tail

            wob = None
            for p in range(NPAIR):
                for nb in range(QNB):
                    run_iteration(p, nb, vproj=(p == 0 and nb == 0),
                                  proj_next=(nb == 1 and p + 1 < NPAIR))
                if p + 1 == NPAIR - 1:
                    # kq inputs/weights are dead once proj_pair(3) is emitted;
                    # reuse the space for wob so its DMA hides under p=3
                    kqd_cm.__exit__(None, None, None)
                    wop = ctx.enter_context(tc.tile_pool(name="wop", bufs=1))
                    wof = wop.tile([128, HC, D], BF16, name="wof")
                    wob = [wof[:, i, :] for i in range(HC)]
                    nc.sync.dma_start(out=wof, in_=woT[:, :, :])
            def emit_outproj(nb):
                for dc in range(EC):
                    ps = pp.tile([128, 512], F32, tag="pp", name=f"ops{dc}_{nb}")
                    for hc in range(HC):
                        nc.tensor.matmul(
                            ps[:, :], wob[hc][:, dc * 128:(dc + 1) * 128],
                            xT[hc][:, nb * 512:(nb + 1) * 512],
                            start=(hc == 0), stop=(hc == HC - 1))
                    o_sb = work.tile([128, 512], F32, tag="osb", bufs=2,
                                     name=f"osb{dc}_{nb}")
                    nc.scalar.add(out=o_sb, in_=ps[:, :], add=t_bo[:, dc:dc + 1])
                    nc.sync.dma_start(
                        out=out[dc * 128:(dc + 1) * 128, nb * 512:(nb + 1) * 512],
                        in_=o_sb)

            # nb=0 output projection overlaps the final (p=3) tail work
            emit_outproj(0)
            pending_tail[0]()
            pending_tail[0] = None
            emit_outproj(1)


def _get_program():
    global _PROGRAM
    if _PROGRAM is None:
        _PROGRAM = _build_program()
    return _PROGRAM


def kernel(q, k, v, mask, Wq, bq, Wk, bk, Wv, bv, Wo, bo, _trace=False):
    bf16 = ml_dtypes.bfloat16
    q = np.asarray(q, np.float32)
    k = np.asarray(k, np.float32)
    v = np.asarray(v, np.float32)
    Wq = np.asarray(Wq, np.float32)
    Wk = np.asarray(Wk, np.float32)
    Wv = np.asarray(Wv, np.float32)
    Wo = np.asarray(Wo, np.float32)
    bq = np.asarray(bq, np.float32)
    bk = np.asarray(bk, np.float32)
    bv = np.asarray(bv, np.float32)
    bo = np.asarray(bo, np.float32)
    mask_f = np.asarray(mask).astype(np.float32)

    nc = _get_program()

    # fold the D (or S) axis into [128 partitions, chunk, ...] so every
    # on-chip DMA is one flat contiguous transfer
    def fold_ec(xT, inner):  # [D, N] -> [128, EC, N] (or [128, x, y] views)
        N = xT.shape[1]
        a = np.ascontiguousarray(xT.reshape(EC, 128, N).transpose(1, 0, 2))
        return a.astype(bf16).reshape((128,) + inner)

    # kT: [D, S] -> [128, SNB, EC, 512] (512-col block major)
    kT_b = [np.ascontiguousarray(
        k[b].T.reshape(EC, 128, SNB, 512).transpose(1, 2, 0, 3)).astype(bf16)
        for b in range(B)]
    # vT: [D, S] -> [128, 4, EC, 512] (quarter major)
    vT_b = [np.ascontiguousarray(
        v[b].T.reshape(EC, 128, 4, 512).transpose(1, 2, 0, 3)).astype(bf16)
        for b in range(B)]
    wqT_f = np.ascontiguousarray(Wq.T).astype(np.float32)
    wkT_f = np.ascontiguousarray(Wk.T)
    wvT_f = np.ascontiguousarray(Wv.T)

    in_maps = []
    for c in range(8):
        b, hg, sq = c // 4, (c // 2) % 2, c % 2
        hsl = slice(hg * DH, (hg + 1) * DH)
        in_maps.append({
            "qT": fold_ec(q[b, sq * Q:(sq + 1) * Q, :].T, (EC, Q)),
            "kT": kT_b[b],
            "vT": vT_b[b],
            "maskT": np.ascontiguousarray(
                mask_f[b, 0, sq * Q:(sq + 1) * Q, :].T.reshape(
                    KC, 128, Q).transpose(1, 0, 2)).astype(bf16),
            "wqT": fold_ec(Wq.T[:, hsl], (EC, DH)),
            "wkT": fold_ec(Wk.T[:, hsl], (EC, DH)),
            "wvT": fold_ec(Wv.T[:, hsl], (EC, DH)),
            "woT": np.ascontiguousarray(
                Wo.T[hsl, :].reshape(HC, 128, D).transpose(1, 0, 2)).astype(bf16),
            "bqv": np.ascontiguousarray(bq[hsl].reshape(NPAIR, 128).T),
            "bkv": np.ascontiguousarray(bk[hsl].reshape(NPAIR, 128).T),
            "bov": np.ascontiguousarray(
                ((bo if hg == 0 else np.zeros_like(bo))
                 + Wo[:, hsl] @ bv[hsl]).reshape(EC, 128).T),
        })

    kw = {}
    if _trace:
        kw = dict(trace=True, trace_cores=list(range(8)))
    res = run_bass_kernel_spmd(nc, in_maps, core_ids=list(range(8)), **kw)
    kernel._last_res = res

    outp = np.empty((B, S, D), np.float32)
    for b in range(B):
        for sq in range(2):
            c0 = b * 4 + sq
            c1 = b * 4 + 2 + sq
            outp[b, sq * Q:(sq + 1) * Q, :] = (
                res.results[c0]["out"] + res.results[c1]["out"]).T
    if _trace:
        return outp, res
    return outp


# revision 25
# speedup vs baseline: 1.1974x; 1.1974x over previous
"""Multi-head attention TRN2 kernel (B=2, S=2048, D=1024, H=16).

Sharding (8 cores): B(2) x head-group(2) x query-block(2).
Each core: one batch b, 8 heads, 1024 query rows. The output projection
is a per-head-group partial sum; the host adds the two partials while
gathering (unshard step).

On-chip layout is feature-major: activations are [feature, token] so
every matmul contracts along the partition dim. Host sharding hands each
core pre-transposed contiguous arrays in bf16 (the kernel computes in
bf16 anyway; converting on host removes the on-chip cast phase and
halves HBM traffic).

Softmax: the reference masks scores with -1e-9 (sic), so masked
positions contribute exp(-1e-9) == 1.0f exactly. We multiply the raw
scores by the 0/1 mask in PSUM (masked -> exp(0) == 1, identical
result), so exp(s*m/8) is directly the PV weight; an appended
ones-column on V yields the softmax denominator in the same matmul.
"""

import sys

if "/opt/trn_rl_repo" not in sys.path:
    sys.path.insert(0, "/opt/trn_rl_repo")

import numpy as np
import ml_dtypes

import concourse.bass as bass
import concourse.tile as tile
from concourse import bacc, mybir
from concourse.bass_utils import run_bass_kernel_spmd

F32 = mybir.dt.float32
F32R = mybir.dt.float32r
BF16 = mybir.dt.bfloat16
AF = mybir.ActivationFunctionType
ALU = mybir.AluOpType

B, S, D, H = 2, 2048, 1024, 16
DK = 64
Q = 1024          # query rows per core
DH = 512          # head-group feature dims per core
NPAIR = 4         # head pairs per core
KC = S // 128     # 16 contraction chunks over k tokens
EC = D // 128     # 8 contraction chunks over model dim
QNB = Q // 512    # 2 query n-blocks
SNB = S // 512    # 4 khT n-blocks
HC = DH // 128    # 4 xT partition chunks

_PROGRAM = None


def _build_program():
    nc = bacc.Bacc("TRN2", debug=False, num_devices=8)

    # all host-side arrays are pre-folded to [128 partitions, ...] so every
    # DMA is a flat contiguous 2D transfer (minimal descriptor count)
    qT = nc.dram_tensor("qT", [128, EC, Q], BF16, kind="ExternalInput")
    kT = nc.dram_tensor("kT", [128, SNB, EC, 512], BF16, kind="ExternalInput")
    vT = nc.dram_tensor("vT", [128, 4, EC, 512], BF16, kind="ExternalInput")
    maskT = nc.dram_tensor("maskT", [128, KC, Q], BF16, kind="ExternalInput")
    wqT = nc.dram_tensor("wqT", [128, EC, DH], BF16, kind="ExternalInput")
    wkT = nc.dram_tensor("wkT", [128, EC, DH], BF16, kind="ExternalInput")
    wvT = nc.dram_tensor("wvT", [128, EC, DH], BF16, kind="ExternalInput")
    woT = nc.dram_tensor("woT", [128, HC, D], BF16, kind="ExternalInput")
    bqv = nc.dram_tensor("bqv", [128, NPAIR], F32, kind="ExternalInput")
    bkv = nc.dram_tensor("bkv", [128, NPAIR], F32, kind="ExternalInput")
    bov = nc.dram_tensor("bov", [128, EC], F32, kind="ExternalInput")
    out = nc.dram_tensor("out", [D, Q], F32, kind="ExternalOutput")

    with tile.TileContext(nc) as tc:
        _emit(nc, tc, qT, kT, vT, maskT, wqT, wkT, wvT, woT, bqv, bkv, bov, out)
    nc.compile()
    return nc


def _emit(nc, tc, qT, kT, vT, maskT, wqT, wkT, wvT, woT, bqv, bkv, bov, out):
    from contextlib import ExitStack

    ctx = ExitStack()
    with ctx:
        consts = ctx.enter_context(tc.tile_pool(name="consts", bufs=1))
        big = ctx.enter_context(tc.tile_pool(name="big", bufs=1))
        work = ctx.enter_context(tc.tile_pool(name="work", bufs=2))
        pp = ctx.enter_context(tc.tile_pool(name="pp", bufs=2, space="PSUM"))
        pvp = ctx.enter_context(tc.tile_pool(name="pvp", bufs=1, space="PSUM"))
        scp = ctx.enter_context(tc.tile_pool(name="scp", bufs=2, space="PSUM"))
        kqd_cm = tc.tile_pool(name="kqd", bufs=1, side="right")
        kqd = kqd_cm.__enter__()

        # ---- small constants ----
        t_bq = consts.tile([128, NPAIR], F32)
        t_bk = consts.tile([128, NPAIR], F32)
        t_bo = consts.tile([128, EC], F32)
        nc.sync.dma_start(out=t_bq, in_=bqv[:, :])
        nc.sync.dma_start(out=t_bk, in_=bkv[:, :])
        nc.sync.dma_start(out=t_bo, in_=bov[:, :])

        # ---- resident activation tensors ----
        khT = [big.tile([128, S], BF16, name=f"khT{i}") for i in range(NPAIR)]
        qhT = [big.tile([128, Q], BF16, name=f"qhT{i}") for i in range(NPAIR)]
        vh_aug = [big.tile([128, 8, 65], BF16, name=f"vha{i}") for i in range(KC)]
        mbf = [big.tile([128, 4, Q], BF16, name=f"mbf{i}") for i in range(4)]
        mb = [mbf[i // 4][:, i % 4, :] for i in range(KC)]

        # ---- k/q weights + inputs: few BIG multi-dim DMAs (queue issue
        # time, ~0.6us per dma_start, was the startup limiter) ----
        kTf = kqd.tile([128, SNB, EC, 512], BF16, name="kTf")
        qTf = kqd.tile([128, EC, Q], BF16, name="qTf")
        wkf = kqd.tile([128, EC, DH], BF16, name="wkf")
        wqf = kqd.tile([128, EC, DH], BF16, name="wqf")
        wkb = [wkf[:, i, :] for i in range(EC)]
        wqb = [wqf[:, i, :] for i in range(EC)]

        nc.sync.dma_start(out=wkf, in_=wkT[:, :, :])
        # kT loaded in 512-col blocks so the first khps proj tile
        # unblocks after ~1MB
        for sb in range(SNB):
            nc.sync.dma_start(out=kTf[:, sb], in_=kT[:, sb])
        nc.sync.dma_start(out=wqf, in_=wqT[:, :, :])
        nc.sync.dma_start(out=qTf, in_=qT[:, :, :])

        def proj_pair(p):
            for sb in range(SNB):
                ps = pp.tile([128, 512], F32, tag="pp", name=f"khps{p}_{sb}")
                for ec in range(EC):
                    nc.tensor.matmul(
                        ps[:, :], wkb[ec][:, p * 128:(p + 1) * 128],
                        kTf[:, sb, ec, :],
                        start=(ec == 0), stop=(ec == EC - 1))
                nc.vector.tensor_scalar(
                    out=khT[p][:, sb * 512:(sb + 1) * 512], in0=ps[:, :],
                    scalar1=t_bk[:, p:p + 1], scalar2=None, op0=ALU.add)
            for qb in range(QNB):
                ps = pp.tile([128, 512], F32, tag="pp", name=f"qhps{p}_{qb}")
                for ec in range(EC):
                    nc.tensor.matmul(
                        ps[:, :], wqb[ec][:, p * 128:(p + 1) * 128],
                        qTf[:, ec, qb * 512:(qb + 1) * 512],
                        start=(ec == 0), stop=(ec == EC - 1))
                nc.vector.tensor_scalar(
                    out=qhT[p][:, qb * 512:(qb + 1) * 512], in0=ps[:, :],
                    scalar1=t_bq[:, p:p + 1], scalar2=None, op0=ALU.add)

        proj_pair(0)

        # ---- masks: 4 consolidated bf16 DMAs on the sync queue ----
        for g in range(4):
            nc.sync.dma_start(out=mbf[g], in_=maskT[:, 4 * g:4 * (g + 1), :])

        # ---- vh projection setup (interleaved with first attention pass) ----
        vs = ctx.enter_context(tc.tile_pool(name="vs", bufs=2))
        vtbp = ctx.enter_context(tc.tile_pool(name="vtb", bufs=1))
        wvf = vtbp.tile([128, EC, DH], BF16, name="wvf")
        wvb = [wvf[:, i, :] for i in range(EC)]
        nc.gpsimd.dma_start(out=wvf, in_=wvT[:, :, :])
        for sc in range(KC):
            nc.vector.memset(vh_aug[sc][:, :, 64:65], 1.0)

        def emit_vproj_qtr(qtr):
            vqf = vs.tile([128, EC, 512], BF16, tag="vq", name=f"vq{qtr}")
            vq = [vqf[:, i, :] for i in range(EC)]
            nc.gpsimd.dma_start(out=vqf, in_=vT[:, qtr])
            for si in range(4):
                sc = qtr * 4 + si
                ps = pp.tile([128, 512], F32, tag="pp", name=f"vps{sc}")
                for ec in range(EC):
                    nc.tensor.matmul(
                        ps[:, :], vq[ec][:, si * 128:(si + 1) * 128], wvb[ec][:, :],
                        start=(ec == 0), stop=(ec == EC - 1))
                nc.scalar.copy(
                    out=vh_aug[sc][:, :, 0:64],
                    in_=ps.rearrange("p (h d) -> p h d", h=8))

        # ---- attention ----
        xT = [big.tile([128, Q], BF16, name=f"xT{i}") for i in range(HC)]
        LAG = 2
        if True:
            pending_tail = [None]

            def run_iteration(p, nb, vproj=False):
                pv0 = pvp.tile([65, 512], F32, tag="pv0", name=f"pv0_{p}{nb}")
                pv1 = pvp.tile([65, 512], F32, tag="pv1", name=f"pv1_{p}{nb}")
                Etiles = {}

                def emit_scores(kc):
                    sc_ps = scp.tile([128, 2, 512], F32, tag="sc",
                                     name=f"sc_{p}_{nb}_{kc}")
                    nc.tensor.matmul(
                        sc_ps[:, 0, :],
                        khT[p][0:64, kc * 128:(kc + 1) * 128],
                        qhT[p][0:64, nb * 512:(nb + 1) * 512],
                        start=True, stop=True)
                    nc.tensor.matmul(
                        sc_ps[:, 1, :],
                        khT[p][64:128, kc * 128:(kc + 1) * 128],
                        qhT[p][64:128, nb * 512:(nb + 1) * 512],
                        start=True, stop=True, tile_position=(64, 0))
                    # mask in place: masked scores -> 0 -> exp(0) == 1,
                    # matching the reference's exp(-1e-9) == 1.0f
                    m_in = mb[kc][:, None, nb * 512:(nb + 1) * 512].to_broadcast(
                        [128, 2, 512])
                    nc.vector.tensor_tensor(
                        out=sc_ps[:, :, :], in0=sc_ps[:, :, :], in1=m_in,
                        op=ALU.mult)
                    E = work.tile([128, 2, 512], BF16, tag="E", bufs=3,
                                  name=f"E{p}_{nb}_{kc}")
                    nc.scalar.activation(
                        out=E.rearrange("p h q -> p (h q)"),
                        in_=sc_ps.rearrange("p h q -> p (h q)"),
                        func=AF.Exp, scale=0.125)
                    Etiles[kc] = E

                def emit_pv(kc):
                    E = Etiles.pop(kc)
                    for hh in range(2):
                        nc.tensor.matmul(
                            (pv0, pv1)[hh][:, :], vh_aug[kc][:, 2 * p + hh, :],
                            E[:, hh, :],
                            start=(kc == 0), stop=(kc == KC - 1))

                for kc in range(KC + LAG):
                    if vproj and kc < KC and kc % 4 == 0:
                        emit_vproj_qtr(kc // 4)
                    if kc < KC:
                        emit_scores(kc)
                    if kc == 3 and pending_tail[0] is not None:
                        # previous iteration's tail drops into the DVE/GpSimd
                        # queues BEHIND this iteration's first mask ops
                        pending_tail[0]()
                        pending_tail[0] = None
                    if kc >= LAG:
                        emit_pv(kc - LAG)

                # evacuate PSUM immediately (ACT) so the pv banks free fast;
                # the rest of the tail is deferred into the next iteration
                pvns = []
                for hh, pv in ((0, pv0), (1, pv1)):
                    pvn = work.tile([65, 512], F32, tag="pvn", bufs=2,
                                    name=f"pvn{p}{nb}{hh}")
                    nc.scalar.copy(out=pvn, in_=pv[:, :])
                    pvns.append(pvn)

                def tail():
                    rcps = []
                    for hh in range(2):
                        den = work.tile([1, 512], F32, tag="den", bufs=2,
                                        name=f"den{p}{nb}{hh}")
                        nc.vector.tensor_copy(out=den, in_=pvns[hh][64:65, :])
                        rcp_f = work.tile([1, 512], F32, tag="rcpf", bufs=2,
                                          name=f"rcpf{p}{nb}{hh}")
                        nc.vector.reciprocal_approx_fast(out=rcp_f, in_=den)
                        rcps.append(rcp_f)
                    brs = []
                    for hh in range(2):
                        br_sb = work.tile([64, 512], F32, tag="brs", bufs=2,
                                          name=f"brs{p}{nb}{hh}")
                        nc.gpsimd.partition_broadcast(br_sb, rcps[hh])
                        brs.append(br_sb)
                    for hh in range(2):
                        h = 2 * p + hh
                        nc.vector.tensor_tensor(
                            out=xT[h // 2][(h % 2) * 64:(h % 2) * 64 + 64,
                                           nb * 512:(nb + 1) * 512],
                            in0=pvns[hh][0:64, :], in1=brs[hh], op=ALU.mult)

                pending_tail[0] = tail

            wob = None
            for p in range(NPAIR):
                for nb in range(QNB):
                    run_iteration(p, nb, vproj=(p == 0 and nb == 0))
                if p + 1 < NPAIR:
                    proj_pair(p + 1)
                if p + 1 == NPAIR - 1:
                    # kq inputs/weights are dead once proj_pair(3) is emitted;
                    # reuse the space for wob so its DMA hides under p=3
                    kqd_cm.__exit__(None, None, None)
                    wop = ctx.enter_context(tc.tile_pool(name="wop", bufs=1))
                    wof = wop.tile([128, HC, D], BF16, name="wof")
                    wob = [wof[:, i, :] for i in range(HC)]
                    nc.sync.dma_start(out=wof, in_=woT[:, :, :])
            def emit_outproj(nb):
                for dc in range(EC):
                    ps = pp.tile([128, 512], F32, tag="pp", name=f"ops{dc}_{nb}")
                    for hc in range(HC):
                        nc.tensor.matmul(
                            ps[:, :], wob[hc][:, dc * 128:(dc + 1) * 128],
                            xT[hc][:, nb * 512:(nb + 1) * 512],
                            start=(hc == 0), stop=(hc == HC - 1))
                    o_sb = work.tile([128, 512], F32, tag="osb", bufs=2,
                                     name=f"osb{dc}_{nb}")
                    nc.scalar.add(out=o_sb, in_=ps[:, :], add=t_bo[:, dc:dc + 1])
                    nc.sync.dma_start(
                        out=out[dc * 128:(dc + 1) * 128, nb * 512:(nb + 1) * 512],
                        in_=o_sb)

            # nb=0 output projection overlaps the final (p=3) tail work
            emit_outproj(0)
            pending_tail[0]()
            pending_tail[0] = None
            emit_outproj(1)


def _get_program():
    global _PROGRAM
    if _PROGRAM is None:
        _PROGRAM = _build_program()
    return _PROGRAM


def kernel(q, k, v, mask, Wq, bq, Wk, bk, Wv, bv, Wo, bo, _trace=False):
    bf16 = ml_dtypes.bfloat16
    q = np.asarray(q, np.float32)
    k = np.asarray(k, np.float32)
    v = np.asarray(v, np.float32)
    Wq = np.asarray(Wq, np.float32)
    Wk = np.asarray(Wk, np.float32)
    Wv = np.asarray(Wv, np.float32)
    Wo = np.asarray(Wo, np.float32)
    bq = np.asarray(bq, np.float32)
    bk = np.asarray(bk, np.float32)
    bv = np.asarray(bv, np.float32)
    bo = np.asarray(bo, np.float32)
    mask_f = np.asarray(mask).astype(np.float32)

    nc = _get_program()

    # fold the D (or S) axis into [128 partitions, chunk, ...] so every
    # on-chip DMA is one flat contiguous transfer
    def fold_ec(xT, inner):  # [D, N] -> [128, EC, N] (or [128, x, y] views)
        N = xT.shape[1]
        a = np.ascontiguousarray(xT.reshape(EC, 128, N).transpose(1, 0, 2))
        return a.astype(bf16).reshape((128,) + inner)

    # kT: [D, S] -> [128, SNB, EC, 512] (512-col block major)
    kT_b = [np.ascontiguousarray(
        k[b].T.reshape(EC, 128, SNB, 512).transpose(1, 2, 0, 3)).astype(bf16)
        for b in range(B)]
    # vT: [D, S] -> [128, 4, EC, 512] (quarter major)
    vT_b = [np.ascontiguousarray(
        v[b].T.reshape(EC, 128, 4, 512).transpose(1, 2, 0, 3)).astype(bf16)
        for b in range(B)]
    wqT_f = np.ascontiguousarray(Wq.T).astype(np.float32)
    wkT_f = np.ascontiguousarray(Wk.T)
    wvT_f = np.ascontiguousarray(Wv.T)

    in_maps = []
    for c in range(8):
        b, hg, sq = c // 4, (c // 2) % 2, c % 2
        hsl = slice(hg * DH, (hg + 1) * DH)
        in_maps.append({
            "qT": fold_ec(q[b, sq * Q:(sq + 1) * Q, :].T, (EC, Q)),
            "kT": kT_b[b],
            "vT": vT_b[b],
            "maskT": np.ascontiguousarray(
                mask_f[b, 0, sq * Q:(sq + 1) * Q, :].T.reshape(
                    KC, 128, Q).transpose(1, 0, 2)).astype(bf16),
            "wqT": fold_ec(Wq.T[:, hsl], (EC, DH)),
            "wkT": fold_ec(Wk.T[:, hsl], (EC, DH)),
            "wvT": fold_ec(Wv.T[:, hsl], (EC, DH)),
            "woT": np.ascontiguousarray(
                Wo.T[hsl, :].reshape(HC, 128, D).transpose(1, 0, 2)).astype(bf16),
            "bqv": np.ascontiguousarray(bq[hsl].reshape(NPAIR, 128).T),
            "bkv": np.ascontiguousarray(bk[hsl].reshape(NPAIR, 128).T),
            "bov": np.ascontiguousarray(
                ((bo if hg == 0 else np.zeros_like(bo))
                 + Wo[:, hsl] @ bv[hsl]).reshape(EC, 128).T),
        })

    kw = {}
    if _trace:
        kw = dict(trace=True, trace_cores=list(range(8)))
    res = run_bass_kernel_spmd(nc, in_maps, core_ids=list(range(8)), **kw)
    kernel._last_res = res

    outp = np.empty((B, S, D), np.float32)
    for b in range(B):
        for sq in range(2):
            c0 = b * 4 + sq
            c1 = b * 4 + 2 + sq
            outp[b, sq * Q:(sq + 1) * Q, :] = (
                res.results[c0]["out"] + res.results[c1]["out"]).T
    if _trace:
        return outp, res
    return outp


# revision 26
# speedup vs baseline: 1.2037x; 1.0053x over previous
"""Multi-head attention TRN2 kernel (B=2, S=2048, D=1024, H=16).

Sharding (8 cores): B(2) x head-group(2) x query-block(2).
Each core: one batch b, 8 heads, 1024 query rows. The output projection
is a per-head-group partial sum; the host adds the two partials while
gathering (unshard step).

On-chip layout is feature-major: activations are [feature, token] so
every matmul contracts along the partition dim. Host sharding hands each
core pre-transposed contiguous arrays in bf16 (the kernel computes in
bf16 anyway; converting on host removes the on-chip cast phase and
halves HBM traffic).

Softmax: the reference masks scores with -1e-9 (sic), so masked
positions contribute exp(-1e-9) == 1.0f exactly. We multiply the raw
scores by the 0/1 mask in PSUM (masked -> exp(0) == 1, identical
result), so exp(s*m/8) is directly the PV weight; an appended
ones-column on V yields the softmax denominator in the same matmul.
"""

import sys

if "/opt/trn_rl_repo" not in sys.path:
    sys.path.insert(0, "/opt/trn_rl_repo")

import numpy as np
import ml_dtypes

import concourse.bass as bass
import concourse.tile as tile
from concourse import bacc, mybir
from concourse.bass_utils import run_bass_kernel_spmd

F32 = mybir.dt.float32
F32R = mybir.dt.float32r
BF16 = mybir.dt.bfloat16
AF = mybir.ActivationFunctionType
ALU = mybir.AluOpType

B, S, D, H = 2, 2048, 1024, 16
DK = 64
Q = 1024          # query rows per core
DH = 512          # head-group feature dims per core
NPAIR = 4         # head pairs per core
KC = S // 128     # 16 contraction chunks over k tokens
EC = D // 128     # 8 contraction chunks over model dim
QNB = Q // 512    # 2 query n-blocks
SNB = S // 512    # 4 khT n-blocks
HC = DH // 128    # 4 xT partition chunks

_PROGRAM = None


def _build_program():
    nc = bacc.Bacc("TRN2", debug=False, num_devices=8)

    # all host-side arrays are pre-folded to [128 partitions, ...] so every
    # DMA is a flat contiguous 2D transfer (minimal descriptor count)
    qT = nc.dram_tensor("qT", [128, EC, Q], BF16, kind="ExternalInput")
    kT = nc.dram_tensor("kT", [128, SNB, EC, 512], BF16, kind="ExternalInput")
    vT = nc.dram_tensor("vT", [128, 4, EC, 512], BF16, kind="ExternalInput")
    maskT = nc.dram_tensor("maskT", [128, KC, Q], BF16, kind="ExternalInput")
    wqT = nc.dram_tensor("wqT", [128, EC, DH], BF16, kind="ExternalInput")
    wkT = nc.dram_tensor("wkT", [128, EC, DH], BF16, kind="ExternalInput")
    wvT = nc.dram_tensor("wvT", [128, EC, DH], BF16, kind="ExternalInput")
    woT = nc.dram_tensor("woT", [128, HC, D], BF16, kind="ExternalInput")
    bqv = nc.dram_tensor("bqv", [128, NPAIR], F32, kind="ExternalInput")
    bkv = nc.dram_tensor("bkv", [128, NPAIR], F32, kind="ExternalInput")
    bov = nc.dram_tensor("bov", [128, EC], F32, kind="ExternalInput")
    out = nc.dram_tensor("out", [D, Q], F32, kind="ExternalOutput")

    with tile.TileContext(nc) as tc:
        _emit(nc, tc, qT, kT, vT, maskT, wqT, wkT, wvT, woT, bqv, bkv, bov, out)
    nc.compile()
    return nc


def _emit(nc, tc, qT, kT, vT, maskT, wqT, wkT, wvT, woT, bqv, bkv, bov, out):
    from contextlib import ExitStack

    ctx = ExitStack()
    with ctx:
        consts = ctx.enter_context(tc.tile_pool(name="consts", bufs=1))
        big = ctx.enter_context(tc.tile_pool(name="big", bufs=1))
        work = ctx.enter_context(tc.tile_pool(name="work", bufs=2))
        pp = ctx.enter_context(tc.tile_pool(name="pp", bufs=2, space="PSUM"))
        pvp = ctx.enter_context(tc.tile_pool(name="pvp", bufs=1, space="PSUM"))
        scp = ctx.enter_context(tc.tile_pool(name="scp", bufs=2, space="PSUM"))
        kqd_cm = tc.tile_pool(name="kqd", bufs=1, side="right")
        kqd = kqd_cm.__enter__()

        # ---- small constants ----
        t_bq = consts.tile([128, NPAIR], F32)
        t_bk = consts.tile([128, NPAIR], F32)
        t_bo = consts.tile([128, EC], F32)
        nc.gpsimd.dma_start(out=t_bq, in_=bqv[:, :])
        nc.gpsimd.dma_start(out=t_bk, in_=bkv[:, :])
        nc.gpsimd.dma_start(out=t_bo, in_=bov[:, :])

        # ---- resident activation tensors ----
        khT = [big.tile([128, S], BF16, name=f"khT{i}") for i in range(NPAIR)]
        qhT = [big.tile([128, Q], BF16, name=f"qhT{i}") for i in range(NPAIR)]
        vh_aug = [big.tile([128, 8, 65], BF16, name=f"vha{i}") for i in range(KC)]
        mbf = [big.tile([128, 4, Q], BF16, name=f"mbf{i}") for i in range(4)]
        mb = [mbf[i // 4][:, i % 4, :] for i in range(KC)]

        vs2 = ctx.enter_context(tc.tile_pool(name="vs", bufs=2))
        vtbp = ctx.enter_context(tc.tile_pool(name="vtb", bufs=1))
        wvf2 = vtbp.tile([128, EC, DH], BF16, name="wvf")
        wvb = [wvf2[:, i, :] for i in range(EC)]

        # ---- k/q weights + inputs: few BIG multi-dim DMAs (queue issue
        # time, ~0.6us per dma_start, was the startup limiter) ----
        kTf = kqd.tile([128, SNB, EC, 512], BF16, name="kTf")
        qTf = kqd.tile([128, EC, Q], BF16, name="qTf")
        wkf = kqd.tile([128, EC, DH], BF16, name="wkf")
        wqf = kqd.tile([128, EC, DH], BF16, name="wqf")
        wkb = [wkf[:, i, :] for i in range(EC)]
        wqb = [wqf[:, i, :] for i in range(EC)]

        nc.sync.dma_start(out=wkf, in_=wkT[:, :, :])
        # kT loaded in 512-col blocks so the first khps proj tile
        # unblocks after ~1MB
        for sb in range(SNB):
            nc.sync.dma_start(out=kTf[:, sb], in_=kT[:, sb])
        nc.sync.dma_start(out=wqf, in_=wqT[:, :, :])
        nc.sync.dma_start(out=qTf, in_=qT[:, :, :])
        # remaining inputs in need-order: first mask group, V weights, then
        # V quarters interleaved with the other mask groups (transfers
        # serialize on the shared SDMA rings, so queue order IS arrival order)
        nc.sync.dma_start(out=mbf[0], in_=maskT[:, 0:4, :])
        nc.sync.dma_start(out=wvf2, in_=wvT[:, :, :])
        vqs = [vs2.tile([128, EC, 512], BF16, tag="vq", name=f"vq{t}")
               for t in range(4)]
        nc.sync.dma_start(out=vqs[0], in_=vT[:, 0])
        for g in range(1, 4):
            nc.sync.dma_start(out=vqs[g], in_=vT[:, g])
            nc.sync.dma_start(out=mbf[g], in_=maskT[:, 4 * g:4 * (g + 1), :])

        def proj_pair(p):
            for sb in range(SNB):
                ps = pp.tile([128, 512], F32, tag="pp", name=f"khps{p}_{sb}")
                for ec in range(EC):
                    nc.tensor.matmul(
                        ps[:, :], wkb[ec][:, p * 128:(p + 1) * 128],
                        kTf[:, sb, ec, :],
                        start=(ec == 0), stop=(ec == EC - 1))
                nc.vector.tensor_scalar(
                    out=khT[p][:, sb * 512:(sb + 1) * 512], in0=ps[:, :],
                    scalar1=t_bk[:, p:p + 1], scalar2=None, op0=ALU.add)
            for qb in range(QNB):
                ps = pp.tile([128, 512], F32, tag="pp", name=f"qhps{p}_{qb}")
                for ec in range(EC):
                    nc.tensor.matmul(
                        ps[:, :], wqb[ec][:, p * 128:(p + 1) * 128],
                        qTf[:, ec, qb * 512:(qb + 1) * 512],
                        start=(ec == 0), stop=(ec == EC - 1))
                nc.vector.tensor_scalar(
                    out=qhT[p][:, qb * 512:(qb + 1) * 512], in0=ps[:, :],
                    scalar1=t_bq[:, p:p + 1], scalar2=None, op0=ALU.add)

        proj_pair(0)


        # ---- vh projection setup (interleaved with first attention pass) ----
        for sc in range(KC):
            nc.vector.memset(vh_aug[sc][:, :, 64:65], 1.0)

        def emit_vproj_qtr(qtr):
            vqf = vqs[qtr]
            vq = [vqf[:, i, :] for i in range(EC)]
            for si in range(4):
                sc = qtr * 4 + si
                ps = pp.tile([128, 512], F32, tag="pp", name=f"vps{sc}")
                for ec in range(EC):
                    nc.tensor.matmul(
                        ps[:, :], vq[ec][:, si * 128:(si + 1) * 128], wvb[ec][:, :],
                        start=(ec == 0), stop=(ec == EC - 1))
                nc.scalar.copy(
                    out=vh_aug[sc][:, :, 0:64],
                    in_=ps.rearrange("p (h d) -> p h d", h=8))

        # ---- attention ----
        xT = [big.tile([128, Q], BF16, name=f"xT{i}") for i in range(HC)]
        LAG = 2
        if True:
            pending_tail = [None]

            def run_iteration(p, nb, vproj=False):
                pv0 = pvp.tile([65, 512], F32, tag="pv0", name=f"pv0_{p}{nb}")
                pv1 = pvp.tile([65, 512], F32, tag="pv1", name=f"pv1_{p}{nb}")
                Etiles = {}

                def emit_scores(kc):
                    sc_ps = scp.tile([128, 2, 512], F32, tag="sc",
                                     name=f"sc_{p}_{nb}_{kc}")
                    nc.tensor.matmul(
                        sc_ps[:, 0, :],
                        khT[p][0:64, kc * 128:(kc + 1) * 128],
                        qhT[p][0:64, nb * 512:(nb + 1) * 512],
                        start=True, stop=True)
                    nc.tensor.matmul(
                        sc_ps[:, 1, :],
                        khT[p][64:128, kc * 128:(kc + 1) * 128],
                        qhT[p][64:128, nb * 512:(nb + 1) * 512],
                        start=True, stop=True, tile_position=(64, 0))
                    # mask in place: masked scores -> 0 -> exp(0) == 1,
                    # matching the reference's exp(-1e-9) == 1.0f
                    m_in = mb[kc][:, None, nb * 512:(nb + 1) * 512].to_broadcast(
                        [128, 2, 512])
                    nc.vector.tensor_tensor(
                        out=sc_ps[:, :, :], in0=sc_ps[:, :, :], in1=m_in,
                        op=ALU.mult)
                    E = work.tile([128, 2, 512], BF16, tag="E", bufs=3,
                                  name=f"E{p}_{nb}_{kc}")
                    nc.scalar.activation(
                        out=E.rearrange("p h q -> p (h q)"),
                        in_=sc_ps.rearrange("p h q -> p (h q)"),
                        func=AF.Exp, scale=0.125)
                    Etiles[kc] = E

                def emit_pv(kc):
                    E = Etiles.pop(kc)
                    for hh in range(2):
                        nc.tensor.matmul(
                            (pv0, pv1)[hh][:, :], vh_aug[kc][:, 2 * p + hh, :],
                            E[:, hh, :],
                            start=(kc == 0), stop=(kc == KC - 1))

                for kc in range(KC + LAG):
                    if vproj and kc < KC and kc % 4 == 0:
                        emit_vproj_qtr(kc // 4)
                    if kc < KC:
                        emit_scores(kc)
                    if kc == 3 and pending_tail[0] is not None:
                        # previous iteration's tail drops into the DVE/GpSimd
                        # queues BEHIND this iteration's first mask ops
                        pending_tail[0]()
                        pending_tail[0] = None
                    if kc >= LAG:
                        emit_pv(kc - LAG)

                # evacuate PSUM immediately (ACT) so the pv banks free fast;
                # the rest of the tail is deferred into the next iteration
                pvns = []
                for hh, pv in ((0, pv0), (1, pv1)):
                    pvn = work.tile([65, 512], F32, tag="pvn", bufs=2,
                                    name=f"pvn{p}{nb}{hh}")
                    nc.scalar.copy(out=pvn, in_=pv[:, :])
                    pvns.append(pvn)

                def tail():
                    rcps = []
                    for hh in range(2):
                        den = work.tile([1, 512], F32, tag="den", bufs=2,
                                        name=f"den{p}{nb}{hh}")
                        nc.vector.tensor_copy(out=den, in_=pvns[hh][64:65, :])
                        rcp_f = work.tile([1, 512], F32, tag="rcpf", bufs=2,
                                          name=f"rcpf{p}{nb}{hh}")
                        nc.vector.reciprocal_approx_fast(out=rcp_f, in_=den)
                        rcps.append(rcp_f)
                    brs = []
                    for hh in range(2):
                        br_sb = work.tile([64, 512], F32, tag="brs", bufs=2,
                                          name=f"brs{p}{nb}{hh}")
                        nc.gpsimd.partition_broadcast(br_sb, rcps[hh])
                        brs.append(br_sb)
                    for hh in range(2):
                        h = 2 * p + hh
                        nc.vector.tensor_tensor(
                            out=xT[h // 2][(h % 2) * 64:(h % 2) * 64 + 64,
                                           nb * 512:(nb + 1) * 512],
                            in0=pvns[hh][0:64, :], in1=brs[hh], op=ALU.mult)

                pending_tail[0] = tail

            wob = None
            for p in range(NPAIR):
                for nb in range(QNB):
                    run_iteration(p, nb, vproj=(p == 0 and nb == 0))
                if p + 1 < NPAIR:
                    proj_pair(p + 1)
                if p + 1 == NPAIR - 1:
                    # kq inputs/weights are dead once proj_pair(3) is emitted;
                    # reuse the space for wob so its DMA hides under p=3
                    kqd_cm.__exit__(None, None, None)
                    wop = ctx.enter_context(tc.tile_pool(name="wop", bufs=1))
                    wof = wop.tile([128, HC, D], BF16, name="wof")
                    wob = [wof[:, i, :] for i in range(HC)]
                    nc.sync.dma_start(out=wof, in_=woT[:, :, :])
            def emit_outproj(nb):
                for dc in range(EC):
                    ps = pp.tile([128, 512], F32, tag="pp", name=f"ops{dc}_{nb}")
                    for hc in range(HC):
                        nc.tensor.matmul(
                            ps[:, :], wob[hc][:, dc * 128:(dc + 1) * 128],
                            xT[hc][:, nb * 512:(nb + 1) * 512],
                            start=(hc == 0), stop=(hc == HC - 1))
                    o_sb = work.tile([128, 512], F32, tag="osb", bufs=2,
                                     name=f"osb{dc}_{nb}")
                    nc.scalar.add(out=o_sb, in_=ps[:, :], add=t_bo[:, dc:dc + 1])
                    nc.sync.dma_start(
                        out=out[dc * 128:(dc + 1) * 128, nb * 512:(nb + 1) * 512],
                        in_=o_sb)

            # nb=0 output projection overlaps the final (p=3) tail work
            emit_outproj(0)
            pending_tail[0]()
            pending_tail[0] = None
            emit_outproj(1)


def _get_program():
    global _PROGRAM
    if _PROGRAM is None:
        _PROGRAM = _build_program()
    return _PROGRAM


def kernel(q, k, v, mask, Wq, bq, Wk, bk, Wv, bv, Wo, bo, _trace=False):
    bf16 = ml_dtypes.bfloat16
    q = np.asarray(q, np.float32)
    k = np.asarray(k, np.float32)
    v = np.asarray(v, np.float32)
    Wq = np.asarray(Wq, np.float32)
    Wk = np.asarray(Wk, np.float32)
    Wv = np.asarray(Wv, np.float32)
    Wo = np.asarray(Wo, np.float32)
    bq = np.asarray(bq, np.float32)
    bk = np.asarray(bk, np.float32)
    bv = np.asarray(bv, np.float32)
    bo = np.asarray(bo, np.float32)
    mask_f = np.asarray(mask).astype(np.float32)

    nc = _get_program()

    # fold the D (or S) axis into [128 partitions, chunk, ...] so every
    # on-chip DMA is one flat contiguous transfer
    def fold_ec(xT, inner):  # [D, N] -> [128, EC, N] (or [128, x, y] views)
        N = xT.shape[1]
        a = np.ascontiguousarray(xT.reshape(EC, 128, N).transpose(1, 0, 2))
        return a.astype(bf16).reshape((128,) + inner)

    # kT: [D, S] -> [128, SNB, EC, 512] (512-col block major)
    kT_b = [np.ascontiguousarray(
        k[b].T.reshape(EC, 128, SNB, 512).transpose(1, 2, 0, 3)).astype(bf16)
        for b in range(B)]
    # vT: [D, S] -> [128, 4, EC, 512] (quarter major)
    vT_b = [np.ascontiguousarray(
        v[b].T.reshape(EC, 128, 4, 512).transpose(1, 2, 0, 3)).astype(bf16)
        for b in range(B)]
    wqT_f = np.ascontiguousarray(Wq.T).astype(np.float32)
    wkT_f = np.ascontiguousarray(Wk.T)
    wvT_f = np.ascontiguousarray(Wv.T)

    in_maps = []
    for c in range(8):
        b, hg, sq = c // 4, (c // 2) % 2, c % 2
        hsl = slice(hg * DH, (hg + 1) * DH)
        in_maps.append({
            "qT": fold_ec(q[b, sq * Q:(sq + 1) * Q, :].T, (EC, Q)),
            "kT": kT_b[b],
            "vT": vT_b[b],
            "maskT": np.ascontiguousarray(
                mask_f[b, 0, sq * Q:(sq + 1) * Q, :].T.reshape(
                    KC, 128, Q).transpose(1, 0, 2)).astype(bf16),
            "wqT": fold_ec(Wq.T[:, hsl], (EC, DH)),
            "wkT": fold_ec(Wk.T[:, hsl], (EC, DH)),
            "wvT": fold_ec(Wv.T[:, hsl], (EC, DH)),
            "woT": np.ascontiguousarray(
                Wo.T[hsl, :].reshape(HC, 128, D).transpose(1, 0, 2)).astype(bf16),
            "bqv": np.ascontiguousarray(bq[hsl].reshape(NPAIR, 128).T),
            "bkv": np.ascontiguousarray(bk[hsl].reshape(NPAIR, 128).T),
            "bov": np.ascontiguousarray(
                ((bo if hg == 0 else np.zeros_like(bo))
                 + Wo[:, hsl] @ bv[hsl]).reshape(EC, 128).T),
        })

    kw = {}
    if _trace:
        kw = dict(trace=True, trace_cores=list(range(8)))
    res = run_bass_kernel_spmd(nc, in_maps, core_ids=list(range(8)), **kw)
    kernel._last_res = res

    outp = np.empty((B, S, D), np.float32)
    for b in range(B):
        for sq in range(2):
            c0 = b * 4 + sq
            c1 = b * 4 + 2 + sq
            outp[b, sq * Q:(sq + 1) * Q, :] = (
                res.results[c0]["out"] + res.results[c1]["out"]).T
    if _trace:
        return outp, res
    return outp
